# revision 4
# baseline (speedup 1.0000x reference)
"""EnergyBasedVAD Trainium2 kernel.

Input:  waveform (32, 960000) f32.
Output: (32, 3749) bool VAD mask.

Sharding: pure data parallel — 4 batch rows per core across 8 cores.

Device computes 256-sample block sums of x^2/512 (the memory-bound part:
123 MB of waveform reads). The HWDGE fast path (~460 GB/s vs ~150 GB/s)
requires exactly 128 partitions, line >= stride (contiguous HBM union)
and partition stride < 32 KiB, so the core's 15,000 blocks are loaded as
4 unchunked dense spans [128, L*256] with L = {30, 30, 30, 28}: 15,104
block-columns — the minimum 128-partition block-aligned cover (0.69%
over the ideal bytes; the 104-block overrun reads zero padding). Each
span is squared on ACT with the 1/512 mean folded into the activation
scale, block-summed 64 -> 256 on DVE into one [128, 118] tile, and the
result leaves in a single dense 60 KB store.

Host reassembles blocks, forms frame energies as adjacent block pairs
(the same fp32 adds the device would do), then the 20%-quantile
threshold and the hysteresis segment state machine — 0.01% of the bytes.
"""

import math
import numpy as np

import concourse.bass as bass
import concourse.bacc as bacc
import concourse.mybir as mybir
from concourse.bass_utils import run_bass_kernel_spmd
from concourse.tile import TileContext

N_CORES = 8
B, S = 32, 960000
ROWS = B // N_CORES          # 4 rows per core
NBLK_ROW = S // 256          # 3750 blocks of 256 per row
NBLK = ROWS * NBLK_ROW       # 15000 blocks per core
T = (S - 512) // 256 + 1     # 3749 output frames
SPAN_L = [30, 30, 30, 28]    # blocks per partition per span (sum*128 = 15104)
NCOL = sum(SPAN_L)           # 118 block columns of real data
NCOLP = 128                  # padded to 512B DRAM lines for the result store
FLAT = 128 * NCOL * 256      # 3,866,624 samples per core incl. 26,624 pad
P = 128

SILENCE_FRAMES = 18
MIN_SPEECH_FRAMES = 6
ENERGY_THRESHOLD = 0.01

_CACHE = {}


def _build(repeat: int = 1):
    nc = bacc.Bacc(None)
    wav = nc.declare_dram_parameter("waveform", [FLAT], mybir.dt.float32, isOutput=False)
    eout = nc.declare_dram_parameter("energy", [P, NCOLP], mybir.dt.float32, isOutput=True)

    inv = 1.0 / math.sqrt(512.0)
    sq_t = mybir.ActivationFunctionType.Square

    with TileContext(nc) as tc:
        with (
            tc.tile_pool(name="wav", bufs=2) as wav_pool,
            tc.tile_pool(name="sq", bufs=2) as sq_pool,
            tc.tile_pool(name="blk", bufs=2) as blk_pool,
        ):
            for _ in range(repeat):
                blks = blk_pool.tile([P, NCOLP], mybir.dt.float32)
                nc.vector.memset(blks[:, NCOL:NCOLP], 0.0)
                base = 0   # span base, samples
                col = 0    # output column base
                for si, L in enumerate(SPAN_L):
                    seg = L * 256
                    # dense span: partition p <- flat[base + p*seg : +seg].
                    # All loads ride the SP ring; ACT only computes, so
                    # descriptor issue never queues behind an activation.
                    wt = wav_pool.tile([P, seg], mybir.dt.float32)
                    nc.sync.dma_start(out=wt[:], in_=bass.AP(wav, base, [[seg, P], [1, seg]]))

                    # bf16 squares halve ACT writes + DVE reads; the f32
                    # accumulate in the reduce keeps block sums accurate
                    sq = sq_pool.tile([P, seg], mybir.dt.bfloat16)
                    nc.scalar.activation(sq[:], wt[:], sq_t, scale=inv)

                    # fused 64 -> 4 block-sum tree in one DVE pass
                    nc.vector.reduce_sum(
                        blks[:, col:col + L],
                        sq[:].rearrange("p (n f g) -> p n f g", f=4, g=64),
                        axis=mybir.AxisListType.XY,
                    )
                    base += P * seg
                    col += L
                # single dense 64 KB store, off both HWDGE rings so neither
                # the load stream nor ACT ever waits on DVE completion
                nc.gpsimd.dma_start(out=eout[:, :], in_=blks[:])
    nc.finalize()   # Bacc: runs the bacc compile pipeline (wait splitting, regalloc)
    return nc


def _in_maps(waveform: np.ndarray):
    w = np.ascontiguousarray(waveform, dtype=np.float32).reshape(N_CORES, ROWS * S)
    pad = np.zeros((N_CORES, FLAT - ROWS * S), np.float32)
    wp = np.concatenate([w, pad], axis=1)
    return [{"waveform": wp[c]} for c in range(N_CORES)]


def _blocks_from_eout(e: np.ndarray) -> np.ndarray:
    """[P, NCOLP] span-packed block sums -> flat (NBLK,) global block sums."""
    blocks = np.empty(P * NCOL, np.float32)
    b0 = 0
    c0 = 0
    for L in SPAN_L:
        blocks[b0 + np.arange(P)[:, None] * L + np.arange(L)[None, :]] = e[:, c0:c0 + L]
        b0 += P * L
        c0 += L
    return blocks[:NBLK]


def _run_device(waveform: np.ndarray, trace: bool = False):
    if "nc" not in _CACHE:
        _CACHE["nc"] = _build()
    nc = _CACHE["nc"]
    res = run_bass_kernel_spmd(nc, _in_maps(waveform), core_ids=list(range(N_CORES)), trace=trace)
    blks = np.stack(
        [_blocks_from_eout(res.results[c]["energy"]).reshape(ROWS, NBLK_ROW)
         for c in range(N_CORES)]
    ).reshape(B, NBLK_ROW)
    energy = blks[:, :T] + blks[:, 1:T + 1]   # frame t = block t + block t+1
    return energy, res


def _vad_from_energy(e: np.ndarray) -> np.ndarray:
    """Threshold + hysteresis state machine, faithful to the reference."""
    n = e.shape[1]
    out = np.zeros((e.shape[0], n), dtype=bool)
    for b in range(e.shape[0]):
        s = np.sort(e[b])
        nzero = int((s <= 0).sum())
        nz = n - nzero
        if nz > 0:
            pos = np.float32(0.2) * np.float32(nz - 1)
            lo = int(np.floor(pos))
            hi = int(np.ceil(pos))
            frac = np.float32(pos) - np.float32(lo)
            ilo = min(max(nzero + lo, 0), n - 1)
            ihi = min(max(nzero + hi, 0), n - 1)
            thr = np.float32(s[ilo] * (np.float32(1.0) - frac) + s[ihi] * frac)
        else:
            thr = np.float32(ENERGY_THRESHOLD)
        m = e[b] > thr
        t = np.nonzero(m)[0]
        if len(t) == 0:
            continue
        grp = np.concatenate([[0], (np.diff(t) > SILENCE_FRAMES).cumsum()])
        for g in range(grp[-1] + 1):
            tg = t[grp == g]
            first, last = int(tg[0]), int(tg[-1])
            if last >= n - SILENCE_FRAMES:
                st, en = first, n      # trailing open segment
            else:
                st, en = first, last   # closed: end excludes last speech frame
            if en - st >= MIN_SPEECH_FRAMES:
                out[b, st:en] = True
    return out


def kernel(waveform: np.ndarray, _trace: bool = False) -> np.ndarray:
    energy, res = _run_device(waveform, trace=_trace)
    _CACHE["last_result"] = res
    return _vad_from_energy(energy)


# ---------------- timing utilities (test-only, not used by kernel()) ----------


def _prepare_call(nc, in_maps):
    """Compile + stage device-resident args; returns a nullary timed callable."""
    import time
    import jax
    from jax.sharding import Mesh, PartitionSpec
    from jax.experimental.shard_map import shard_map
    from concourse import bass2jax

    bass2jax.install_neuronx_cc_hook()
    n_cores = len(in_maps)
    part_name = nc.partition_id_tensor.name if nc.partition_id_tensor else None
    in_names, out_names, out_avals, zero_outs = [], [], [], []
    for alloc in nc.m.functions[0].allocations:
        if not isinstance(alloc, mybir.MemoryLocationSet):
            continue
        name = alloc.memorylocations[0].name
        if alloc.kind == "ExternalInput":
            if name != part_name:
                in_names.append(name)
        elif alloc.kind == "ExternalOutput":
            shape = tuple(alloc.tensor_shape)
            dtype = mybir.dt.np(alloc.dtype)
            out_names.append(name)
            out_avals.append(jax.core.ShapedArray(shape, dtype))
            zero_outs.append(np.zeros(shape, dtype))
    n_params = len(in_names)
    all_in_names = in_names + out_names
    if part_name is not None:
        all_in_names = all_in_names + [part_name]

    def _body(*args):
        operands = list(args)
        if part_name is not None:
            operands.append(bass2jax.partition_id_tensor())
        return tuple(bass2jax._bass_exec_p.bind(
            *operands,
            out_avals=tuple(out_avals), in_names=tuple(all_in_names),
            out_names=tuple(out_names), lowering_input_output_aliases=(),
            sim_require_finite=True, sim_require_nnan=True, nc=nc,
        ))

    devices = jax.devices()[:n_cores]
    mesh = Mesh(np.asarray(devices), ("core",))
    fn = jax.jit(shard_map(
        _body, mesh=mesh,
        in_specs=(PartitionSpec("core"),) * (n_params + len(out_names)),
        out_specs=(PartitionSpec("core"),) * len(out_names),
        check_rep=False,
    ))
    sharding = jax.sharding.NamedSharding(mesh, PartitionSpec("core"))
    args = [
        jax.device_put(np.concatenate([np.asarray(in_maps[c][n]) for c in range(n_cores)], 0), sharding)
        for n in in_names
    ] + [
        jax.device_put(np.zeros((n_cores * z.shape[0], *z.shape[1:]), z.dtype), sharding)
        for z in zero_outs
    ]

    def call():
        t0 = time.perf_counter()
        jax.block_until_ready(fn(*args))
        return time.perf_counter() - t0
    return call


def measure_exec_ns(repeat: int = 257, rounds: int = 4, iters: int = 8, verbose: bool = True):
    """Estimate HW kernel time by differencing an N-repeat program against the
    N=1 program. Measurement rounds are interleaved A/B to cancel the slow
    drift of the tunnel dispatch overhead; min-of-all per executable."""
    w = _CACHE.get("timing_input")
    if w is None:
        w = np.random.default_rng(0).standard_normal((B, S), dtype=np.float32)
    maps = _in_maps(w)
    call1 = _prepare_call(_CACHE.setdefault("nc", _build()), maps)
    callR = _prepare_call(_CACHE.setdefault(f"nc_rep{repeat}", _build(repeat)), maps)
    call1(); callR()  # warm both (NEFF load)
    t1, tR = [], []
    for _ in range(rounds):
        t1 += [call1() for _ in range(iters)]
        tR += [callR() for _ in range(iters)]
    min1, minR = min(t1), min(tR)
    ns = (minR - min1) / (repeat - 1) * 1e9
    if verbose:
        print(f"  [timing] per-call wall min: N=1 {min1*1e3:.2f}ms, N={repeat} {minR*1e3:.2f}ms"
              f" -> body {ns:.0f} ns")
    return ns


# revision 5
# speedup vs baseline: 4.9947x; 4.9947x over previous
"""EnergyBasedVAD Trainium2 kernel.

Input:  waveform (32, 960000) f32.
Output: (32, 3749) bool VAD mask.

Sharding: pure data parallel — 4 batch rows per core across 8 cores.

Device computes 256-sample block sums of x^2/512 (the memory-bound part:
123 MB of waveform reads). The HWDGE fast path (~460 GB/s vs ~150 GB/s)
requires exactly 128 partitions, line >= stride (contiguous HBM union)
and partition stride < 32 KiB, so the core's 15,000 blocks are loaded as
4 unchunked dense spans [128, L*256] with L = {30, 30, 30, 28}: 15,104
block-columns — the minimum 128-partition block-aligned cover (0.69%
over the ideal bytes; the 104-block overrun reads zero padding). Each
span is squared on ACT with the 1/512 mean folded into the activation
scale, block-summed 64 -> 256 on DVE into one [128, 118] tile, and the
result leaves in a single dense 60 KB store.

Host reassembles blocks, forms frame energies as adjacent block pairs
(the same fp32 adds the device would do), then the 20%-quantile
threshold and the hysteresis segment state machine — 0.01% of the bytes.
"""

import math
import numpy as np

import concourse.bass as bass
import concourse.bacc as bacc
import concourse.mybir as mybir
from concourse.bass_utils import run_bass_kernel_spmd
from concourse.tile import TileContext

N_CORES = 8
B, S = 32, 960000
ROWS = B // N_CORES          # 4 rows per core
NBLK_ROW = S // 256          # 3750 blocks of 256 per row
NBLK = ROWS * NBLK_ROW       # 15000 blocks per core
T = (S - 512) // 256 + 1     # 3749 output frames
SPAN_L = [30, 30, 30, 28]    # blocks per partition per span (sum*128 = 15104)
NCOL = sum(SPAN_L)           # 118 block columns of real data
NCOLP = 128                  # padded to 512B DRAM lines for the result store
FLAT = 128 * NCOL * 256      # 3,866,624 samples per core incl. 26,624 pad
P = 128

SILENCE_FRAMES = 18
MIN_SPEECH_FRAMES = 6
ENERGY_THRESHOLD = 0.01

_CACHE = {}


def _build(repeat: int = 1):
    nc = bacc.Bacc(None)
    wav = nc.declare_dram_parameter("waveform", [FLAT], mybir.dt.float32, isOutput=False)
    eout = nc.declare_dram_parameter("energy", [P, NCOLP], mybir.dt.float32, isOutput=True)

    sq_t = mybir.ActivationFunctionType.Square

    with TileContext(nc) as tc:
        with (
            tc.tile_pool(name="wav", bufs=3) as wav_pool,
            tc.tile_pool(name="sq", bufs=2) as sq_pool,
            tc.tile_pool(name="blk", bufs=2) as blk_pool,
        ):
            for _ in range(repeat):
                blks = blk_pool.tile([P, NCOLP], mybir.dt.float32)
                nc.vector.memset(blks[:, NCOL:NCOLP], 0.0)
                base = 0   # span base, samples
                col = 0    # output column base
                for si, L in enumerate(SPAN_L):
                    seg = L * 256
                    # dense span: partition p <- flat[base + p*seg : +seg].
                    # All loads ride the SP ring; ACT only computes, so
                    # descriptor issue never queues behind an activation.
                    wt = wav_pool.tile([P, seg], mybir.dt.float32)
                    nc.sync.dma_start(out=wt[:], in_=bass.AP(wav, base, [[seg, P], [1, seg]]))

                    # bf16 squares halve ACT writes + DVE reads; the f32
                    # accumulate in the reduce keeps block sums accurate.
                    # ACT and DVE split the squaring ~70/30 so neither engine
                    # runs ahead of the load stream; the 1/512 mean factor is
                    # folded into one post-scale of the block sums.
                    a = (int(seg * 0.7) // 256) * 256
                    sq = sq_pool.tile([P, seg], mybir.dt.bfloat16)
                    nc.scalar.activation(sq[:, :a], wt[:, :a], sq_t)
                    nc.vector.tensor_mul(sq[:, a:], wt[:, a:], wt[:, a:])

                    # fused 64 -> 4 block-sum tree in one DVE pass
                    nc.vector.reduce_sum(
                        blks[:, col:col + L],
                        sq[:].rearrange("p (n f g) -> p n f g", f=4, g=64),
                        axis=mybir.AxisListType.XY,
                    )
                    base += P * seg
                    col += L
                nc.vector.tensor_scalar_mul(blks[:, :NCOL], blks[:, :NCOL], 1.0 / 512.0)
                # single dense 64 KB store, off both HWDGE rings so neither
                # the load stream nor ACT ever waits on DVE completion
                nc.gpsimd.dma_start(out=eout[:, :], in_=blks[:])
    nc.finalize()   # Bacc: runs the bacc compile pipeline (wait splitting, regalloc)
    return nc


def _in_maps(waveform: np.ndarray):
    w = np.ascontiguousarray(waveform, dtype=np.float32).reshape(N_CORES, ROWS * S)
    pad = np.zeros((N_CORES, FLAT - ROWS * S), np.float32)
    wp = np.concatenate([w, pad], axis=1)
    return [{"waveform": wp[c]} for c in range(N_CORES)]


def _blocks_from_eout(e: np.ndarray) -> np.ndarray:
    """[P, NCOLP] span-packed block sums -> flat (NBLK,) global block sums."""
    blocks = np.empty(P * NCOL, np.float32)
    b0 = 0
    c0 = 0
    for L in SPAN_L:
        blocks[b0 + np.arange(P)[:, None] * L + np.arange(L)[None, :]] = e[:, c0:c0 + L]
        b0 += P * L
        c0 += L
    return blocks[:NBLK]


def _run_device(waveform: np.ndarray, trace: bool = False):
    if "nc" not in _CACHE:
        _CACHE["nc"] = _build()
    nc = _CACHE["nc"]
    res = run_bass_kernel_spmd(nc, _in_maps(waveform), core_ids=list(range(N_CORES)), trace=trace)
    blks = np.stack(
        [_blocks_from_eout(res.results[c]["energy"]).reshape(ROWS, NBLK_ROW)
         for c in range(N_CORES)]
    ).reshape(B, NBLK_ROW)
    energy = blks[:, :T] + blks[:, 1:T + 1]   # frame t = block t + block t+1
    return energy, res


def _vad_from_energy(e: np.ndarray) -> np.ndarray:
    """Threshold + hysteresis state machine, faithful to the reference."""
    n = e.shape[1]
    out = np.zeros((e.shape[0], n), dtype=bool)
    for b in range(e.shape[0]):
        s = np.sort(e[b])
        nzero = int((s <= 0).sum())
        nz = n - nzero
        if nz > 0:
            pos = np.float32(0.2) * np.float32(nz - 1)
            lo = int(np.floor(pos))
            hi = int(np.ceil(pos))
            frac = np.float32(pos) - np.float32(lo)
            ilo = min(max(nzero + lo, 0), n - 1)
            ihi = min(max(nzero + hi, 0), n - 1)
            thr = np.float32(s[ilo] * (np.float32(1.0) - frac) + s[ihi] * frac)
        else:
            thr = np.float32(ENERGY_THRESHOLD)
        m = e[b] > thr
        t = np.nonzero(m)[0]
        if len(t) == 0:
            continue
        grp = np.concatenate([[0], (np.diff(t) > SILENCE_FRAMES).cumsum()])
        for g in range(grp[-1] + 1):
            tg = t[grp == g]
            first, last = int(tg[0]), int(tg[-1])
            if last >= n - SILENCE_FRAMES:
                st, en = first, n      # trailing open segment
            else:
                st, en = first, last   # closed: end excludes last speech frame
            if en - st >= MIN_SPEECH_FRAMES:
                out[b, st:en] = True
    return out


def kernel(waveform: np.ndarray, _trace: bool = False) -> np.ndarray:
    energy, res = _run_device(waveform, trace=_trace)
    _CACHE["last_result"] = res
    return _vad_from_energy(energy)


# ---------------- timing utilities (test-only, not used by kernel()) ----------


def _prepare_call(nc, in_maps):
    """Compile + stage device-resident args; returns a nullary timed callable."""
    import time
    import jax
    from jax.sharding import Mesh, PartitionSpec
    from jax.experimental.shard_map import shard_map
    from concourse import bass2jax

    bass2jax.install_neuronx_cc_hook()
    n_cores = len(in_maps)
    part_name = nc.partition_id_tensor.name if nc.partition_id_tensor else None
    in_names, out_names, out_avals, zero_outs = [], [], [], []
    for alloc in nc.m.functions[0].allocations:
        if not isinstance(alloc, mybir.MemoryLocationSet):
            continue
        name = alloc.memorylocations[0].name
        if alloc.kind == "ExternalInput":
            if name != part_name:
                in_names.append(name)
        elif alloc.kind == "ExternalOutput":
            shape = tuple(alloc.tensor_shape)
            dtype = mybir.dt.np(alloc.dtype)
            out_names.append(name)
            out_avals.append(jax.core.ShapedArray(shape, dtype))
            zero_outs.append(np.zeros(shape, dtype))
    n_params = len(in_names)
    all_in_names = in_names + out_names
    if part_name is not None:
        all_in_names = all_in_names + [part_name]

    def _body(*args):
        operands = list(args)
        if part_name is not None:
            operands.append(bass2jax.partition_id_tensor())
        return tuple(bass2jax._bass_exec_p.bind(
            *operands,
            out_avals=tuple(out_avals), in_names=tuple(all_in_names),
            out_names=tuple(out_names), lowering_input_output_aliases=(),
            sim_require_finite=True, sim_require_nnan=True, nc=nc,
        ))

    devices = jax.devices()[:n_cores]
    mesh = Mesh(np.asarray(devices), ("core",))
    fn = jax.jit(shard_map(
        _body, mesh=mesh,
        in_specs=(PartitionSpec("core"),) * (n_params + len(out_names)),
        out_specs=(PartitionSpec("core"),) * len(out_names),
        check_rep=False,
    ))
    sharding = jax.sharding.NamedSharding(mesh, PartitionSpec("core"))
    args = [
        jax.device_put(np.concatenate([np.asarray(in_maps[c][n]) for c in range(n_cores)], 0), sharding)
        for n in in_names
    ] + [
        jax.device_put(np.zeros((n_cores * z.shape[0], *z.shape[1:]), z.dtype), sharding)
        for z in zero_outs
    ]

    def call():
        t0 = time.perf_counter()
        jax.block_until_ready(fn(*args))
        return time.perf_counter() - t0
    return call


def measure_exec_ns(repeat: int = 257, rounds: int = 4, iters: int = 8, verbose: bool = True):
    """Estimate HW kernel time by differencing an N-repeat program against the
    N=1 program. Measurement rounds are interleaved A/B to cancel the slow
    drift of the tunnel dispatch overhead; min-of-all per executable."""
    w = _CACHE.get("timing_input")
    if w is None:
        w = np.random.default_rng(0).standard_normal((B, S), dtype=np.float32)
    maps = _in_maps(w)
    call1 = _prepare_call(_CACHE.setdefault("nc", _build()), maps)
    callR = _prepare_call(_CACHE.setdefault(f"nc_rep{repeat}", _build(repeat)), maps)
    call1(); callR()  # warm both (NEFF load)
    t1, tR = [], []
    for _ in range(rounds):
        t1 += [call1() for _ in range(iters)]
        tR += [callR() for _ in range(iters)]
    min1, minR = min(t1), min(tR)
    ns = (minR - min1) / (repeat - 1) * 1e9
    if verbose:
        print(f"  [timing] per-call wall min: N=1 {min1*1e3:.2f}ms, N={repeat} {minR*1e3:.2f}ms"
              f" -> body {ns:.0f} ns")
    return ns
